# revision 29
# baseline (speedup 1.0000x reference)
"""Trainium2 Bass kernel for nn_BackProject: batched bilinear sampling.

reference: out[b, d, h, w, c] = bilinear_sample(inputs[b], coords[b, d, h, w])
  inputs [2, 120, 160, 32] f32, coords [2, 32, 120, 160, 2] f32 (x, y),
  out [2, 32, 120, 160, 32] f32.

Sharding: 64 (b, d) planes / 8 cores = 8 planes per core; cores 0-3 take
b=0, cores 4-7 take b=1.  Each core holds the full [H, W, C] feature map
(as a host-prebuilt quad table).

v2 design (vs the v1 on-device-quad-table f32 kernel):
  * The "quad table" qt[p] = pixels [p, p+1, p+W, p+W+1] is built on the
    HOST in bf16 (pure layout/precision staging, no arithmetic), so each
    bilinear sample is ONE 256-byte dma_gather descriptor instead of 512.
    The gather path is DMA-descriptor-rate-bound, so halving descriptor
    bytes and skipping the on-device build shortens the critical path.
  * Gather indices y0*W+x0 are computed on DVE in f32 (exact) as in v1.
  * Bilinear weights for all 8 planes are computed in ONE batched pass
    (16 [128, 1200] DVE ops instead of 160 per-plane ops) and stored
    interleaved as w4[p, j, k] bf16, k in (w00, w01, w10, w11).
  * The lerp is 3 DVE ops per half-plane on bf16 (2x DVE rate):
      mul:  gt[p, t, k, c] *= w4[p, j(t), k] (free-dim broadcast over c)
      add1: gt[.., 0:2, ..] += gt[.., 2:4, ..]
      add2: out = gt[.., 0, ..] + gt[.., 1, ..]
  * Output is stored bf16 (halves store traffic); the host upcasts to
    f32.  bf16 keeps worst-case rel err ~0.5% << the 2e-2 gate.
"""

import sys

for _p in ("/opt/trn_rl_repo", "/opt/pypackages"):
    if _p not in sys.path:
        sys.path.append(_p)

import numpy as np

B, H, W, C = 2, 120, 160, 32
D = 32
P = H * W            # 19200 positions per plane
PLANES = 8           # planes per core
S = 75               # positions per partition per half-plane
QROWS = P - W - 1    # 19039 valid quad rows (max gathered idx is 19038)
FQ = 19040           # quad-table rows incl. one pad row
MAGIC = 12582912.0   # 1.5 * 2**23: x + MAGIC - MAGIC == rne(x) for 0<=x<2**22
SINGLE_PACKET = False

_cache = {}


def _split_multi_waits(nc):
    """The pinned walrus build accepts only one sync-wait per instruction;
    Tile aggregates several.  Hoist all but the last wait of every
    instruction onto same-engine NOPs inserted right before it."""
    import concourse.mybir as mybir

    for bb in nc.main_func.blocks:
        lst = bb.instructions
        snapshot = list(lst)
        if not any(
            i.sync_info is not None and i.sync_info.on_wait and len(i.sync_info.on_wait) > 1
            for i in snapshot
        ):
            continue
        rebuilt = []
        for inst in snapshot:
            si = inst.sync_info
            if si is not None and si.on_wait and len(si.on_wait) > 1:
                waits = list(si.on_wait)
                eng = nc.engines[inst.engine]
                for w in waits[:-1]:
                    nop = eng.nop().ins
                    # nop() appended itself somewhere; pull it out
                    for bb2 in nc.main_func.blocks:
                        l2 = bb2.instructions
                        if l2 and l2[-1] is nop:
                            l2.remove(nop)
                            break
                    nop.sync_info = mybir.SyncInfo(on_wait=[w], on_update=[])
                    rebuilt.append(nop)
                si.on_wait = waits[-1:]
            rebuilt.append(inst)
        lst.clear()
        lst.extend(rebuilt)


def _build():
    import concourse.bass as bass
    import concourse.mybir as mybir
    import concourse.tile as tile
    from concourse import library_config
    from concourse.library_overlay import lower_extended_insts
    from bass_rust import add_dep_helper

    f32 = mybir.dt.float32
    bf16 = mybir.dt.bfloat16
    i16 = mybir.dt.int16
    Alu = mybir.AluOpType

    nc = bass.Bass(num_swdge_queues=4)
    qt = nc.dram_tensor("qt", [FQ, 4 * C], bf16, kind="ExternalInput")
    coords = nc.dram_tensor("coords", [PLANES, P, 2], f32, kind="ExternalInput")
    # host-preswizzled copy of coords in the wrapped staging order:
    # coordsw[h, 16d+r, q, tt, e] = coords[d, (16q+r)*150 + 75h + tt, e]
    coordsw = nc.dram_tensor(
        "coordsw", [2, 128, 8, S, 2], f32, kind="ExternalInput"
    )
    out = nc.dram_tensor("out", [PLANES, P, C], bf16, kind="ExternalOutput")

    with tile.TileContext(nc) as tc:
        with tc.tile_pool(name="persist", bufs=1) as pers:
            ll = nc.gpsimd.load_library(library_config.mlp)
            n_gathers = 0
            v = nc.vector

            # constant per-partition columns, broadcast along the free dim in
            # tensor_tensor ops (TENSOR_SCALAR measures ~10x slower than TT
            # on this silicon/ucode combination)
            cmagic = pers.tile([128, 1], f32)
            cone = pers.tile([128, 1], f32)
            cw_ = pers.tile([128, 1], f32)
            v.memset(cmagic[:], MAGIC)
            v.memset(cone[:], 1.0)
            v.memset(cw_[:], float(W))

            def bc(t, n):
                return t.broadcast_to([128, n])

            pidx = pers.tile([128, PLANES * 1200], i16)
            idx16 = pers.tile([128, 1200], i16)
            # interleaved bilinear weights: w4[p, 150*d + 75*h + tt, k]
            w4 = pers.tile([128, 1200, 4], bf16)

            # --- Phase B: gather indices in wrapped layout -------------------
            # Positions are stripe-major, pos = p*150 + t, so each partition's
            # output slice is one fat contiguous HBM run.  Gather (d, h)
            # list-pos j = tt*128 + p covers pos = p*150 + 75h + tt; its idx
            # sits at wrapped [j%16, j//16] = [r, 8tt+q] with p = 16q+r.
            # Batched over planes: cw[16d+r, 600h+8tt+q, e].
            with tc.tile_pool(name="idxb", bufs=1) as ib:
                # q-major staging layout: cw[16d+r, h, q, tt, e] loads as one
                # fat DMA per (d, h); the 8tt+q wrapped ordering is produced
                # later by a strided DVE copy of the int16 indices.
                cw = ib.tile([128, 2, 8, S, 2], f32)
                for h in range(2):
                    nc.sync.dma_start(cw[:, h], coordsw[h])

                cwf = cw[:].rearrange("p h q t e -> p (h q t) e")
                rx = ib.tile([128, 1200], f32)
                ry = ib.tile([128, 1200], f32)
                gtw = ib.tile([128, 1200], f32)
                pixf = ib.tile([128, 1200], f32)
                idxq = ib.tile([128, 2, 8, S], i16)
                mg = bc(cmagic, 600)
                iv = idx16[:].rearrange("p (h t q) -> p h t q", h=2, t=S, q=8)

                # One full pass per half-plane so the h=0 indices (and their
                # pidx window replications below) are ready ~12us earlier --
                # the first gathers only need half the index set.
                for h in range(2):
                    hs = slice(600 * h, 600 * (h + 1))
                    xw = cwf[:, hs, 0]
                    yw = cwf[:, hs, 1]
                    # floor(x) via round-to-nearest + correction
                    v.tensor_tensor(rx[:, hs], xw, mg, Alu.add)
                    v.tensor_tensor(rx[:, hs], rx[:, hs], mg, Alu.subtract)
                    v.tensor_tensor(gtw[:, hs], rx[:, hs], xw, Alu.is_gt)
                    v.tensor_tensor(rx[:, hs], rx[:, hs], gtw[:, hs], Alu.subtract)
                    v.tensor_tensor(ry[:, hs], yw, mg, Alu.add)
                    v.tensor_tensor(ry[:, hs], ry[:, hs], mg, Alu.subtract)
                    v.tensor_tensor(gtw[:, hs], ry[:, hs], yw, Alu.is_gt)
                    v.tensor_tensor(ry[:, hs], ry[:, hs], gtw[:, hs], Alu.subtract)
                    v.tensor_tensor(pixf[:, hs], ry[:, hs], bc(cw_, 600), Alu.mult)
                    v.tensor_tensor(pixf[:, hs], pixf[:, hs], rx[:, hs], Alu.add)
                    v.tensor_copy(
                        idxq[:, h].rearrange("p q t -> p (q t)"), pixf[:, hs]
                    )
                    # permute (q, tt) -> 8tt+q wrapped order
                    v.tensor_copy(
                        iv[:, h].rearrange("p t q -> p q t"), idxq[:, h]
                    )
                    # replicate this half's indices across the 8 core windows
                    for d in range(PLANES):
                        for g8 in range(8):
                            nc.sync.dma_start(
                                pidx[
                                    16 * g8:16 * (g8 + 1),
                                    1200 * d + 600 * h:1200 * d + 600 * (h + 1),
                                ],
                                idx16[16 * d:16 * (d + 1), hs],
                            )

            # cn[p, d, t, e] = coords[d, 150p + t, e]; j = 150d + t matches
            # the lerp's sample order (partition p, half h=t//75, col t%75).
            # Loaded on the scalar ring so it doesn't sit ahead of the pidx
            # replications in the sync ring.
            cn = pers.tile([128, 8, 150, 2], f32)
            nc.scalar.dma_start(
                cn[:], coords.rearrange("d (p t) e -> p d t e", p=128, t=150)
            )

            def weight_phase():
                # bilinear weights, batched over all planes
                with tc.tile_pool(name="wts", bufs=1) as wt:
                    xv = cn[:, :, :, 0]
                    yv = cn[:, :, :, 1]
                    a = wt.tile([128, 1200], f32)
                    g = wt.tile([128, 1200], f32)
                    wx = wt.tile([128, 1200], f32)
                    wy = wt.tile([128, 1200], f32)
                    mg = bc(cmagic, 1200)
                    one = bc(cone, 1200)

                    v.tensor_tensor(a[:], xv, mg, Alu.add)
                    v.tensor_tensor(a[:], a[:], mg, Alu.subtract)
                    v.tensor_tensor(g[:], a[:], xv, Alu.is_gt)
                    v.tensor_tensor(a[:], a[:], g[:], Alu.subtract)
                    v.tensor_tensor(wx[:], xv, a[:], Alu.subtract)
                    v.tensor_tensor(a[:], yv, mg, Alu.add)
                    v.tensor_tensor(a[:], a[:], mg, Alu.subtract)
                    v.tensor_tensor(g[:], a[:], yv, Alu.is_gt)
                    v.tensor_tensor(a[:], a[:], g[:], Alu.subtract)
                    v.tensor_tensor(wy[:], yv, a[:], Alu.subtract)
                    # a = 1-wx, g = 1-wy
                    v.tensor_tensor(a[:], one, wx[:], Alu.subtract)
                    v.tensor_tensor(g[:], one, wy[:], Alu.subtract)
                    v.tensor_tensor(w4[:, :, 0], a[:], g[:], Alu.mult)
                    v.tensor_tensor(w4[:, :, 1], wx[:], g[:], Alu.mult)
                    v.tensor_tensor(w4[:, :, 2], a[:], wy[:], Alu.mult)
                    v.tensor_tensor(w4[:, :, 3], wx[:], wy[:], Alu.mult)

            # --- Phase C: per-plane gather, lerp, store ----------------------
            # Software-pipelined emission: gathers are emitted LAG tiles ahead
            # of their lerps, so the conservative cross-engine barriers only
            # order gather(t) behind lerp(t-LAG), which is long done.
            with (
                tc.tile_pool(name="g", bufs=4) as gp,
                tc.tile_pool(name="we", bufs=2) as wep,
                tc.tile_pool(name="o", bufs=2) as op_,
            ):
                tiles = [(d, h) for d in range(PLANES) for h in range(2)]
                gts = {}
                # one shared count register per sub-gather size (to_reg
                # allocates a fresh register per call; 48+ would exhaust them)
                nreg = {25: nc.gpsimd.to_reg(25 * 128)}

                def emit_gathers(t):
                    nonlocal n_gathers
                    d, h = tiles[t]
                    gt = gp.tile([128, S, 4, C], bf16, tag="gt")
                    gts[t] = gt
                    gtf = gt[:].rearrange("p t k c -> p t (k c)")
                    # 3 sub-gathers of 25 columns (201 descriptors per
                    # DMA-engine ring slice -- the proven max that fits the
                    # SWDGE ring).  Each queue runs ONE batch per ring-reclaim
                    # round trip with ~9us of fixed dead time (completion-sem
                    # round trip) regardless of batch size, so the largest
                    # batch amortizes it best: ~16us/201 descs vs the
                    # ~13.5us/105 descs of the finer splits.
                    c0 = 0
                    for sg in range(3):
                        cols = 25
                        i0 = 1200 * d + 600 * h + 8 * c0
                        gi = nc.gpsimd.dma_gather(
                            gtf[:, c0:c0 + cols],
                            qt[0:QROWS],
                            pidx[:, i0:i0 + 8 * cols],
                            128 * cols,
                            nreg[cols],
                            4 * C,
                            single_packet=SINGLE_PACKET,
                            queue_num=n_gathers % 4,
                        )
                        n_gathers += 1
                        add_dep_helper(gi.ins, ll.ins, False, "lib first")
                        c0 += cols

                wes = {}

                def emit_copy(t):
                    # expand the per-sample weights over the channel dim on
                    # the (otherwise idle) Act engine: a DVE op with a
                    # stride-0 operand runs at half rate, so feeding the mul
                    # two dense operands buys ~5us/tile of DVE time
                    d, h = tiles[t]
                    wv = (
                        w4[:, 150 * d + S * h:150 * d + S * (h + 1), :]
                        .unsqueeze(3)
                        .broadcast_to([128, S, 4, C])
                    )
                    we = wep.tile([128, S, 4, C], bf16, tag="we")
                    wes[t] = we
                    nc.scalar.copy(we[:], wv)

                def emit_lerp(t):
                    # per-25-col chunks matching the sub-gather batches: each
                    # chunk's mul only waits for ITS batch (Tile tracks
                    # slice-level deps), so the lerp starts when the first
                    # batch lands instead of the third, and the end-of-kernel
                    # backlog shrinks to one chunk (~22us of the measured
                    # 33us tail was lerp work queued behind the last gather).
                    d, h = tiles[t]
                    gt = gts.pop(t)
                    we = wes.pop(t)
                    dst = out[d].rearrange(
                        "(p h t) c -> h p (t c)", p=128, h=2, t=S
                    )
                    for sg in range(3):
                        cs = slice(25 * sg, 25 * (sg + 1))
                        v.tensor_tensor(
                            we[:, cs], gt[:, cs], we[:, cs], Alu.mult
                        )
                        v.tensor_tensor(
                            we[:, cs, 0:2, :], we[:, cs, 0:2, :],
                            we[:, cs, 2:4, :], Alu.add,
                        )
                        ot = op_.tile([128, 25, C], bf16, tag="ot")
                        v.tensor_tensor(
                            ot[:], we[:, cs, 0, :], we[:, cs, 1, :], Alu.add
                        )
                        nc.scalar.dma_start(
                            dst[h, :, C * 25 * sg:C * 25 * (sg + 1)],
                            ot[:].rearrange("p t c -> p (t c)"),
                        )

                n = len(tiles)
                emit_gathers(0)
                emit_gathers(1)
                weight_phase()
                emit_gathers(2)
                emit_gathers(3)
                emit_copy(0)
                emit_copy(1)
                for t in range(n):
                    if t + 2 < n:
                        emit_copy(t + 2)
                    emit_lerp(t)
                    if t + 4 < n:
                        emit_gathers(t + 4)

    _split_multi_waits(nc)
    lower_extended_insts(nc)
    return nc


def _make_in_maps(inputs, coords):
    import ml_dtypes

    bf16 = ml_dtypes.bfloat16
    inputs = np.ascontiguousarray(np.asarray(inputs, dtype=np.float32))
    coords = np.ascontiguousarray(np.asarray(coords, dtype=np.float32))
    qts = []
    for b in range(B):
        fm = inputs[b].reshape(P, C).astype(bf16)
        qtab = np.zeros((FQ, 4 * C), dtype=bf16)
        for k, off in enumerate((0, 1, W, W + 1)):
            qtab[:QROWS, k * C:(k + 1) * C] = fm[off:off + QROWS]
        qts.append(qtab)
    in_maps = []
    for k in range(8):
        b = k // 4
        d0 = 8 * (k % 4)
        cc = np.ascontiguousarray(coords[b, d0:d0 + 8].reshape(PLANES, P, 2))
        # wrapped staging order: [h, 16d+r, q, tt, e]
        cv = cc.reshape(PLANES, 8, 16, 2, S, 2)         # d, q, r, h, tt, e
        cwp = np.ascontiguousarray(
            cv.transpose(3, 0, 2, 1, 4, 5).reshape(2, 128, 8, S, 2)
        )
        in_maps.append({"qt": qts[b], "coords": cc, "coordsw": cwp})
    return in_maps


def kernel(inputs, coords):
    if "nc" not in _cache:
        _cache["nc"] = _build()
    nc = _cache["nc"]

    from concourse.bass_utils import run_bass_kernel_spmd

    in_maps = _make_in_maps(inputs, coords)
    res = run_bass_kernel_spmd(nc, in_maps, core_ids=list(range(8)))

    out = np.empty((B, D, H, W, C), dtype=np.float32)
    for k in range(8):
        b = k // 4
        d0 = 8 * (k % 4)
        out[b, d0:d0 + 8] = (
            np.asarray(res.results[k]["out"])
            .astype(np.float32)
            .reshape(PLANES, H, W, C)
        )
    return out
